# revision 10
# baseline (speedup 1.0000x reference)
"""Distributed Trainium2 (Bass/Tile) kernel for a 16-head attention block.

Problem: x:[2,2048,1024], 16 heads of dim 64, full [B,H,S,S] additive bias,
softmax, out-projection.  Runs SPMD on 8 NeuronCores: mesh = batch(2) x
head-group(4), i.e. each core handles one batch element and 4 heads
(tensor-parallel split of wq/wk/wv columns and wo rows).  Each core emits a
partial [S, D] output; the host sums the 4 head-group partials per batch.

Device-side formulation (per core, heads h=0..3 local):
  QT/KT = (w @ x^T) in [dq, s] layout, duplicated into both partition
          halves so K=64 score matmuls for adjacent k-tiles can run
          CONCURRENTLY on disjoint PE row-groups (tile_position packing).
  PSUM  = bias^T + K Q^T   (bias lands via TensorE identity-matmul for
          even k-tiles and via a VectorE add for odd k-tiles -- static
          load balance between the two engines)
  P^T   = exp(PSUM)                (one wide ScalarE op per [128,1024] tile)
  attnoutT/denoms via one matmul against V augmented with a ones column
  attnT = attnoutT * (1/denom) broadcast    (softmax normalization)
  out_partial = attnT^T @ woT     (TensorE, overlapped with the next
          query-half's attention)
All matmuls in bf16 with f32 PSUM accumulation; softmax math in f32.
"""

import os
import sys

try:
    import concourse  # noqa: F401
except ImportError:  # pragma: no cover - fallback for bare containers
    for _p in ("/opt/trn_rl_repo", os.path.expanduser("~/.axon_site/_ro/trn_rl_repo")):
        if os.path.isdir(_p) and _p not in sys.path:
            sys.path.insert(0, _p)

from contextlib import ExitStack

import ml_dtypes
import numpy as np

import concourse.mybir as mybir
import concourse.tile as tile
from concourse import bacc
from concourse.bass_utils import run_bass_kernel_spmd
from concourse.masks import make_identity

BF16 = ml_dtypes.bfloat16

B, S, D = 2, 2048, 1024
H, HD = 16, 64
NCORES = 8
HG = 4                 # head groups (tensor-parallel factor)
HPG = H // HG          # heads per group = 4
DG = HPG * HD          # feature cols per group = 256
P = 128
KD = D // P            # contraction chunks for projections = 8
ST = S // P            # 128-row tiles along sequence = 16
NQB = S // 512         # 512-wide query blocks = 4

_CACHE = {}


def _build_nc():
    bf = mybir.dt.bfloat16
    f32 = mybir.dt.float32
    Exp = mybir.ActivationFunctionType.Exp
    mult = mybir.AluOpType.mult
    add = mybir.AluOpType.add

    nc = bacc.Bacc("TRN2", target_bir_lowering=False, debug=False,
                   num_devices=NCORES)

    xT = nc.dram_tensor("xT", [D, S], bf, kind="ExternalInput")
    wqT = nc.dram_tensor("wqT", [D, DG], bf, kind="ExternalInput")
    wkT = nc.dram_tensor("wkT", [D, DG], bf, kind="ExternalInput")
    wvT = nc.dram_tensor("wvT", [D, DG], bf, kind="ExternalInput")
    woT = nc.dram_tensor("woT", [DG, D], bf, kind="ExternalInput")
    ebT = nc.dram_tensor("ebT", [HPG, S, S], bf, kind="ExternalInput")
    out = nc.dram_tensor("out", [S, D], f32, kind="ExternalOutput")

    with tile.TileContext(nc) as tc, ExitStack() as ctx:
        const = ctx.enter_context(tc.tile_pool(name="const", bufs=1))
        eb_pool = ctx.enter_context(tc.tile_pool(name="eb", bufs=8))
        pt_pool = ctx.enter_context(tc.tile_pool(name="pt", bufs=6))
        ev_pool = ctx.enter_context(tc.tile_pool(name="ev", bufs=3))
        outsb = ctx.enter_context(tc.tile_pool(name="outsb", bufs=3))

        # ---- persistent SBUF tensors ----
        xT_sb = const.tile([P, KD, S], bf, tag="xT_sb")
        wqT_sb = const.tile([P, KD, DG], bf, tag="wqT_sb")
        wkT_sb = const.tile([P, KD, DG], bf, tag="wkT_sb")
        wvT_sb = const.tile([P, KD, DG], bf, tag="wvT_sb")
        woT_sb = const.tile([P, DG // P, D], bf, tag="woT_sb")
        # per-head Q^T/K^T duplicated into both partition halves
        QT_sb = const.tile([P, HPG, S], bf, tag="QT_sb")
        KT_sb = const.tile([P, HPG, S], bf, tag="KT_sb")
        # V with a ones column appended per head: [s, h*(HD+1)]
        Vg_sb = const.tile([P, ST, HPG * (HD + 1)], bf, tag="Vg_sb")
        attnT_sb = const.tile([P, DG // P, S], bf, tag="attnT_sb")
        ident_sb = const.tile([P, P], bf, tag="ident_sb")

        make_identity(nc, ident_sb[:])

        # split the big input DMAs across engines/queues for a faster ramp
        xTr = xT[:].rearrange("(o p) s -> p o s", p=P)
        for kc in range(KD):
            eng = (nc.sync, nc.gpsimd, nc.scalar)[kc % 3]
            eng.dma_start(out=xT_sb[:, kc, :], in_=xTr[:, kc, :])
        nc.sync.dma_start(out=wqT_sb[:], in_=wqT[:].rearrange("(o p) c -> p o c", p=P))
        nc.gpsimd.dma_start(out=wkT_sb[:], in_=wkT[:].rearrange("(o p) c -> p o c", p=P))
        nc.sync.dma_start(out=wvT_sb[:], in_=wvT[:].rearrange("(o p) c -> p o c", p=P))
        nc.scalar.dma_start(out=woT_sb[:], in_=woT[:].rearrange("(o p) d -> p o d", p=P))

        # ones columns for V augmentation (V copies below overwrite the rest)
        nc.vector.memset(Vg_sb[:], 1.0)

        # ---- projections (own PSUM pool, closed before attention) ----
        with ExitStack() as ph1:
            psum_q = ph1.enter_context(
                tc.tile_pool(name="psum_q", bufs=2, space="PSUM"))
            for w_sb, dst, scale in ((wqT_sb, QT_sb, 0.125), (wkT_sb, KT_sb, None)):
                for m in range(DG // P):
                    for n in range(NQB):
                        ps = psum_q.tile([P, 512], f32, tag="q")
                        for kc in range(KD):
                            nc.tensor.matmul(
                                ps[:],
                                lhsT=w_sb[:, kc, m * P:(m + 1) * P],
                                rhs=xT_sb[:, kc, n * 512:(n + 1) * 512],
                                start=(kc == 0), stop=(kc == KD - 1),
                            )
                        # psum rows 0:64 = head 2m, rows 64:128 = head 2m+1;
                        # write each head's block into BOTH partition halves
                        nsl = slice(n * 512, (n + 1) * 512)
                        for h2 in range(2):
                            src = ps[h2 * HD:(h2 + 1) * HD, :]
                            for half in range(2):
                                dsl = dst[half * HD:(half + 1) * HD, 2 * m + h2, nsl]
                                if scale is None:
                                    nc.vector.tensor_copy(out=dsl, in_=src)
                                else:
                                    nc.vector.tensor_scalar_mul(dsl, src, scale)

            for t in range(ST):
                ps = psum_q.tile([P, DG], f32, tag="q")
                for kc in range(KD):
                    nc.tensor.matmul(
                        ps[:],
                        lhsT=xT_sb[:, kc, t * P:(t + 1) * P],
                        rhs=wvT_sb[:, kc, :],
                        start=(kc == 0), stop=(kc == KD - 1),
                    )
                for h in range(HPG):
                    nc.vector.tensor_copy(
                        out=Vg_sb[:, t, h * (HD + 1):h * (HD + 1) + HD],
                        in_=ps[:, h * HD:(h + 1) * HD])

        # ---- attention + overlapped output projection ----
        with ExitStack() as ph2:
            psum_s = ph2.enter_context(
                tc.tile_pool(name="psum_s", bufs=3, space="PSUM"))
            psum_o = ph2.enter_context(
                tc.tile_pool(name="psum_o", bufs=2, space="PSUM"))

            def attn_pass(h, qbp):
                hp, h2 = divmod(h, 2)
                q0 = qbp * 1024
                po = {}
                for qb2 in range(2):
                    po[qb2] = psum_o.tile(
                        [P, 512], f32, tag="o", name=f"po_{qb2}")
                for ktp in range(ST // 2):
                    ps = {}
                    ebt = {}
                    for u in range(2):          # unit = one k-tile of 128
                        kt = 2 * ktp + u
                        ebt[u] = eb_pool.tile([P, 1024], bf, tag="eb", name=f"ebt_{u}")
                        dma_eng = nc.sync if u == 0 else nc.gpsimd
                        dma_eng.dma_start(
                            out=ebt[u][:],
                            in_=ebT[h, kt * P:(kt + 1) * P, q0:q0 + 1024])
                        ps[u] = psum_s.tile([P, 1024], f32, tag="s", name=f"ps_{u}")
                    # bias via TensorE identity-matmul (group start); one
                    # identity stationary load serves all four matmuls
                    for u in range(2):
                        for qb2 in range(2):
                            nc.tensor.matmul(
                                ps[u][:, qb2 * 512:(qb2 + 1) * 512],
                                lhsT=ident_sb[:],
                                rhs=ebt[u][:, qb2 * 512:(qb2 + 1) * 512],
                                start=True, stop=False,
                            )
                    # scores: unit 0 on PE rows 0-63, unit 1 on rows 64-127,
                    # interleaved so the two K=64 matmuls can run concurrently
                    for qb2 in range(2):
                        for u in range(2):
                            kt = 2 * ktp + u
                            hh = slice(u * HD, (u + 1) * HD)
                            nc.tensor.matmul(
                                ps[u][:, qb2 * 512:(qb2 + 1) * 512],
                                lhsT=KT_sb[hh, h, kt * P:(kt + 1) * P],
                                rhs=QT_sb[hh, h,
                                          q0 + qb2 * 512:q0 + (qb2 + 1) * 512],
                                start=False, stop=True,
                            )
                    pt = {}
                    for u in range(2):
                        pt[u] = pt_pool.tile([P, 1024], bf, tag="pt",
                                             name=f"pt_{u}")
                        nc.scalar.activation(pt[u][:], ps[u][:], Exp)
                    for u in range(2):
                        kt = 2 * ktp + u
                        for qb2 in range(2):
                            nc.tensor.matmul(
                                po[qb2][:HD + 1, :],
                                lhsT=Vg_sb[:, kt,
                                           h * (HD + 1):(h + 1) * (HD + 1)],
                                rhs=pt[u][:, qb2 * 512:(qb2 + 1) * 512],
                                start=(kt == 0), stop=(kt == ST - 1),
                            )
                # evacuate the PV accumulators with single quick copies so
                # the PSUM banks free up for the next pass immediately; the
                # normalization chain then runs off-critical-path from SBUF
                poc = ev_pool.tile([HD, 1024], f32, tag="poc")
                den = ev_pool.tile([1, 1024], f32, tag="den")
                for qb2 in range(2):
                    qsl = slice(qb2 * 512, (qb2 + 1) * 512)
                    nc.vector.tensor_copy(out=poc[:, qsl], in_=po[qb2][:HD, :])
                    # single-channel reads must land on partition 0: pull the
                    # denominator row straight out of PSUM
                    nc.vector.tensor_copy(out=den[:, qsl], in_=po[qb2][HD:HD + 1, :])
                rc = ev_pool.tile([1, 1024], f32, tag="rc")
                nc.vector.reciprocal_approx_fast(out=rc[:], in_=den[:])
                bc = ev_pool.tile([HD, 1024], f32, tag="bc")
                nc.gpsimd.partition_broadcast(bc[:], rc[:])
                nc.vector.tensor_tensor(
                    attnT_sb[h2 * HD:(h2 + 1) * HD, hp, q0:q0 + 1024],
                    poc[:HD, :], bc[:], mult)

            def oproj_half(qbp):
                # out rows for this query half; fat psum tile = both nb halves
                for st in range(qbp * (ST // 2), (qbp + 1) * (ST // 2)):
                    ps = psum_s.tile([P, 1024], f32, tag="s", name="ps_w")
                    for nb in range(2):
                        for c in range(DG // P):
                            nc.tensor.matmul(
                                ps[:, nb * 512:(nb + 1) * 512],
                                lhsT=attnT_sb[:, c, st * P:(st + 1) * P],
                                rhs=woT_sb[:, c, nb * 512:(nb + 1) * 512],
                                start=(c == 0), stop=(c == DG // P - 1),
                            )
                    ob = outsb.tile([P, D], f32, tag="ob")
                    nc.vector.tensor_copy(out=ob[:], in_=ps[:])
                    nc.gpsimd.dma_start(out=out[st * P:(st + 1) * P, :], in_=ob[:])

            for qbp in range(2):
                for h in range(HPG):
                    attn_pass(h, qbp)
                oproj_half(qbp)

    nc.compile()
    return nc


def _get_nc():
    if "nc" not in _CACHE:
        _CACHE["nc"] = _build_nc()
    return _CACHE["nc"]


def kernel(x, mask, attn_bias, wq, wk, wv, wo):
    x = np.asarray(x, dtype=np.float32)
    mask = np.asarray(mask, dtype=np.float32)
    attn_bias = np.asarray(attn_bias, dtype=np.float32)
    wq = np.asarray(wq, dtype=np.float32)
    wk = np.asarray(wk, dtype=np.float32)
    wv = np.asarray(wv, dtype=np.float32)
    wo = np.asarray(wo, dtype=np.float32)

    bias = attn_bias
    if mask.any():
        bias = bias + mask  # broadcast [1,1,S,S] over [B,H,S,S]

    nc = _get_nc()

    in_maps = []
    for core in range(NCORES):
        b, hg = divmod(core, HG)
        c0, c1 = hg * DG, (hg + 1) * DG
        m = {
            "xT": np.ascontiguousarray(x[b].T).astype(BF16),
            "wqT": np.ascontiguousarray(wq[c0:c1, :].T).astype(BF16),
            "wkT": np.ascontiguousarray(wk[c0:c1, :].T).astype(BF16),
            "wvT": np.ascontiguousarray(wv[c0:c1, :].T).astype(BF16),
            "woT": np.ascontiguousarray(wo[:, c0:c1].T).astype(BF16),
            # bias^T per local head: [h, k, q]
            "ebT": np.ascontiguousarray(
                bias[b, hg * HPG:(hg + 1) * HPG].transpose(0, 2, 1)
            ).astype(BF16),
        }
        in_maps.append(m)

    res = run_bass_kernel_spmd(nc, in_maps, core_ids=list(range(NCORES)))

    full = np.zeros((B, S, D), dtype=np.float32)
    for core in range(NCORES):
        b = core // HG
        full[b] += np.asarray(res.results[core]["out"], dtype=np.float32)
    return full


# revision 11
# speedup vs baseline: 1.0742x; 1.0742x over previous
"""Distributed Trainium2 (Bass/Tile) kernel for a 16-head attention block.

Problem: x:[2,2048,1024], 16 heads of dim 64, full [B,H,S,S] additive bias,
softmax, out-projection.  Runs SPMD on 8 NeuronCores: mesh = batch(2) x
head-group(4), i.e. each core handles one batch element and 4 heads
(tensor-parallel split of wq/wk/wv columns and wo rows).  Each core emits a
partial [S, D] output; the host sums the 4 head-group partials per batch.

Device-side formulation (per core, heads h=0..3 local):
  QT/KT = (w @ x^T) in [dq, s] layout, duplicated into both partition
          halves so K=64 score matmuls for adjacent k-tiles can run
          CONCURRENTLY on disjoint PE row-groups (tile_position packing).
  PSUM  = bias^T + K Q^T   (bias lands via TensorE identity-matmul for
          even k-tiles and via a VectorE add for odd k-tiles -- static
          load balance between the two engines)
  P^T   = exp(PSUM)                (one wide ScalarE op per [128,1024] tile)
  attnoutT/denoms via one matmul against V augmented with a ones column
  attnT = attnoutT * (1/denom) broadcast    (softmax normalization)
  out_partial = attnT^T @ woT     (TensorE, overlapped with the next
          query-half's attention)
All matmuls in bf16 with f32 PSUM accumulation; softmax math in f32.
"""

import os
import sys

try:
    import concourse  # noqa: F401
except ImportError:  # pragma: no cover - fallback for bare containers
    for _p in ("/opt/trn_rl_repo", os.path.expanduser("~/.axon_site/_ro/trn_rl_repo")):
        if os.path.isdir(_p) and _p not in sys.path:
            sys.path.insert(0, _p)

from contextlib import ExitStack

import ml_dtypes
import numpy as np

import concourse.mybir as mybir
import concourse.tile as tile
from concourse import bacc
from concourse.bass_utils import run_bass_kernel_spmd
from concourse.masks import make_identity

BF16 = ml_dtypes.bfloat16

B, S, D = 2, 2048, 1024
H, HD = 16, 64
NCORES = 8
HG = 4                 # head groups (tensor-parallel factor)
HPG = H // HG          # heads per group = 4
DG = HPG * HD          # feature cols per group = 256
P = 128
KD = D // P            # contraction chunks for projections = 8
ST = S // P            # 128-row tiles along sequence = 16
NQB = S // 512         # 512-wide query blocks = 4

_CACHE = {}


def _build_nc():
    bf = mybir.dt.bfloat16
    f32 = mybir.dt.float32
    Exp = mybir.ActivationFunctionType.Exp
    mult = mybir.AluOpType.mult
    add = mybir.AluOpType.add

    nc = bacc.Bacc("TRN2", target_bir_lowering=False, debug=False,
                   num_devices=NCORES)

    xT = nc.dram_tensor("xT", [D, S], bf, kind="ExternalInput")
    wqT = nc.dram_tensor("wqT", [D, DG], bf, kind="ExternalInput")
    wkT = nc.dram_tensor("wkT", [D, DG], bf, kind="ExternalInput")
    wvT = nc.dram_tensor("wvT", [D, DG], bf, kind="ExternalInput")
    woT = nc.dram_tensor("woT", [DG, D], bf, kind="ExternalInput")
    ebT = nc.dram_tensor("ebT", [HPG, S, S], bf, kind="ExternalInput")
    out = nc.dram_tensor("out", [S, D], f32, kind="ExternalOutput")

    with tile.TileContext(nc) as tc, ExitStack() as ctx:
        const = ctx.enter_context(tc.tile_pool(name="const", bufs=1))
        eb_pool = ctx.enter_context(tc.tile_pool(name="eb", bufs=8))
        pt_pool = ctx.enter_context(tc.tile_pool(name="pt", bufs=6))
        ev_pool = ctx.enter_context(tc.tile_pool(name="ev", bufs=3))
        outsb = ctx.enter_context(tc.tile_pool(name="outsb", bufs=3))

        # ---- persistent SBUF tensors ----
        xT_sb = const.tile([P, KD, S], bf, tag="xT_sb")
        wqT_sb = const.tile([P, KD, DG], bf, tag="wqT_sb")
        wkT_sb = const.tile([P, KD, DG], bf, tag="wkT_sb")
        wvT_sb = const.tile([P, KD, DG], bf, tag="wvT_sb")
        woT_sb = const.tile([P, DG // P, D], bf, tag="woT_sb")
        # per-head Q^T/K^T duplicated into both partition halves
        QT_sb = const.tile([P, HPG, S], bf, tag="QT_sb")
        KT_sb = const.tile([P, HPG, S], bf, tag="KT_sb")
        # V with a ones column appended per head: [s, h*(HD+1)]
        Vg_sb = const.tile([P, ST, HPG * (HD + 1)], bf, tag="Vg_sb")
        attnT_sb = const.tile([P, DG // P, S], bf, tag="attnT_sb")
        ident_sb = const.tile([P, P], bf, tag="ident_sb")

        make_identity(nc, ident_sb[:])

        # split the big input DMAs across engines/queues for a faster ramp
        xTr = xT[:].rearrange("(o p) s -> p o s", p=P)
        for kc in range(KD):
            eng = (nc.sync, nc.gpsimd, nc.scalar)[kc % 3]
            eng.dma_start(out=xT_sb[:, kc, :], in_=xTr[:, kc, :])
        nc.sync.dma_start(out=wqT_sb[:], in_=wqT[:].rearrange("(o p) c -> p o c", p=P))
        nc.gpsimd.dma_start(out=wkT_sb[:], in_=wkT[:].rearrange("(o p) c -> p o c", p=P))
        nc.sync.dma_start(out=wvT_sb[:], in_=wvT[:].rearrange("(o p) c -> p o c", p=P))
        nc.scalar.dma_start(out=woT_sb[:], in_=woT[:].rearrange("(o p) d -> p o d", p=P))

        # ones columns for V augmentation (V copies below overwrite the rest)
        nc.vector.memset(Vg_sb[:], 1.0)

        # ---- projections (own PSUM pool, closed before attention) ----
        with ExitStack() as ph1:
            psum_q = ph1.enter_context(
                tc.tile_pool(name="psum_q", bufs=4, space="PSUM"))
            for w_sb, dst, scale in ((wqT_sb, QT_sb, 0.125), (wkT_sb, KT_sb, None)):
                for m in range(DG // P):
                    for n in range(NQB):
                        ps = psum_q.tile([P, 512], f32, tag="q")
                        for kc in range(KD):
                            nc.tensor.matmul(
                                ps[:],
                                lhsT=w_sb[:, kc, m * P:(m + 1) * P],
                                rhs=xT_sb[:, kc, n * 512:(n + 1) * 512],
                                start=(kc == 0), stop=(kc == KD - 1),
                            )
                        # psum rows 0:64 = head 2m, rows 64:128 = head 2m+1;
                        # write each head's block into BOTH partition halves
                        nsl = slice(n * 512, (n + 1) * 512)
                        for h2 in range(2):
                            src = ps[h2 * HD:(h2 + 1) * HD, :]
                            for half in range(2):
                                dsl = dst[half * HD:(half + 1) * HD, 2 * m + h2, nsl]
                                if scale is None:
                                    if half == 0:
                                        nc.vector.tensor_copy(out=dsl, in_=src)
                                    else:
                                        nc.scalar.copy(out=dsl, in_=src)
                                else:
                                    if half == 0:
                                        nc.vector.tensor_scalar_mul(dsl, src, scale)
                                    else:
                                        nc.scalar.mul(out=dsl, in_=src, mul=scale)

            for t in range(ST):
                ps = psum_q.tile([P, DG], f32, tag="q")
                for kc in range(KD):
                    nc.tensor.matmul(
                        ps[:],
                        lhsT=xT_sb[:, kc, t * P:(t + 1) * P],
                        rhs=wvT_sb[:, kc, :],
                        start=(kc == 0), stop=(kc == KD - 1),
                    )
                for h in range(HPG):
                    vout = Vg_sb[:, t, h * (HD + 1):h * (HD + 1) + HD]
                    vin = ps[:, h * HD:(h + 1) * HD]
                    if h % 2 == 0:
                        nc.vector.tensor_copy(out=vout, in_=vin)
                    else:
                        nc.scalar.copy(out=vout, in_=vin)

        # ---- attention + overlapped output projection ----
        with ExitStack() as ph2:
            psum_s = ph2.enter_context(
                tc.tile_pool(name="psum_s", bufs=3, space="PSUM"))
            psum_o = ph2.enter_context(
                tc.tile_pool(name="psum_o", bufs=2, space="PSUM"))

            def attn_pass(h, qbp):
                hp, h2 = divmod(h, 2)
                q0 = qbp * 1024
                po = {}
                for qb2 in range(2):
                    po[qb2] = psum_o.tile(
                        [P, 512], f32, tag="o", name=f"po_{qb2}")
                for ktp in range(ST // 2):
                    ps = {}
                    ebt = {}
                    for u in range(2):          # unit = one k-tile of 128
                        kt = 2 * ktp + u
                        ebt[u] = eb_pool.tile([P, 1024], bf, tag="eb", name=f"ebt_{u}")
                        dma_eng = nc.sync if u == 0 else nc.gpsimd
                        dma_eng.dma_start(
                            out=ebt[u][:],
                            in_=ebT[h, kt * P:(kt + 1) * P, q0:q0 + 1024])
                        ps[u] = psum_s.tile([P, 1024], f32, tag="s", name=f"ps_{u}")
                    # bias via TensorE identity-matmul (group start); one
                    # identity stationary load serves all four matmuls
                    for u in range(2):
                        for qb2 in range(2):
                            nc.tensor.matmul(
                                ps[u][:, qb2 * 512:(qb2 + 1) * 512],
                                lhsT=ident_sb[:],
                                rhs=ebt[u][:, qb2 * 512:(qb2 + 1) * 512],
                                start=True, stop=False,
                            )
                    # scores: unit 0 on PE rows 0-63, unit 1 on rows 64-127,
                    # interleaved so the two K=64 matmuls can run concurrently
                    for qb2 in range(2):
                        for u in range(2):
                            kt = 2 * ktp + u
                            hh = slice(u * HD, (u + 1) * HD)
                            nc.tensor.matmul(
                                ps[u][:, qb2 * 512:(qb2 + 1) * 512],
                                lhsT=KT_sb[hh, h, kt * P:(kt + 1) * P],
                                rhs=QT_sb[hh, h,
                                          q0 + qb2 * 512:q0 + (qb2 + 1) * 512],
                                start=False, stop=True,
                                tile_position=(u * HD, 0),
                            )
                    pt = {}
                    for u in range(2):
                        pt[u] = pt_pool.tile([P, 1024], bf, tag="pt",
                                             name=f"pt_{u}")
                        nc.scalar.activation(pt[u][:], ps[u][:], Exp)
                    for u in range(2):
                        kt = 2 * ktp + u
                        for qb2 in range(2):
                            nc.tensor.matmul(
                                po[qb2][:HD + 1, :],
                                lhsT=Vg_sb[:, kt,
                                           h * (HD + 1):(h + 1) * (HD + 1)],
                                rhs=pt[u][:, qb2 * 512:(qb2 + 1) * 512],
                                start=(kt == 0), stop=(kt == ST - 1),
                            )
                # evacuate the PV accumulators with single quick copies so
                # the PSUM banks free up for the next pass immediately; the
                # normalization chain then runs off-critical-path from SBUF
                poc = ev_pool.tile([HD, 1024], f32, tag="poc")
                den = ev_pool.tile([1, 1024], f32, tag="den")
                for qb2 in range(2):
                    qsl = slice(qb2 * 512, (qb2 + 1) * 512)
                    nc.vector.tensor_copy(out=poc[:, qsl], in_=po[qb2][:HD, :])
                    # single-channel reads must land on partition 0: pull the
                    # denominator row straight out of PSUM
                    nc.vector.tensor_copy(out=den[:, qsl], in_=po[qb2][HD:HD + 1, :])
                rc = ev_pool.tile([1, 1024], f32, tag="rc")
                nc.vector.reciprocal_approx_fast(out=rc[:], in_=den[:])
                bc = ev_pool.tile([HD, 1024], f32, tag="bc")
                nc.gpsimd.partition_broadcast(bc[:], rc[:])
                nc.vector.tensor_tensor(
                    attnT_sb[h2 * HD:(h2 + 1) * HD, hp, q0:q0 + 1024],
                    poc[:HD, :], bc[:], mult)

            def oproj_half(qbp):
                # out rows for this query half; fat psum tile = both nb halves
                for st in range(qbp * (ST // 2), (qbp + 1) * (ST // 2)):
                    ps = psum_s.tile([P, 1024], f32, tag="s", name="ps_w")
                    for nb in range(2):
                        for c in range(DG // P):
                            nc.tensor.matmul(
                                ps[:, nb * 512:(nb + 1) * 512],
                                lhsT=attnT_sb[:, c, st * P:(st + 1) * P],
                                rhs=woT_sb[:, c, nb * 512:(nb + 1) * 512],
                                start=(c == 0), stop=(c == DG // P - 1),
                            )
                    ob = outsb.tile([P, D], f32, tag="ob")
                    nc.vector.tensor_copy(out=ob[:], in_=ps[:])
                    nc.gpsimd.dma_start(out=out[st * P:(st + 1) * P, :], in_=ob[:])

            for qbp in range(2):
                for h in range(HPG):
                    attn_pass(h, qbp)
                oproj_half(qbp)

    nc.compile()
    return nc


def _get_nc():
    if "nc" not in _CACHE:
        _CACHE["nc"] = _build_nc()
    return _CACHE["nc"]


def kernel(x, mask, attn_bias, wq, wk, wv, wo):
    x = np.asarray(x, dtype=np.float32)
    mask = np.asarray(mask, dtype=np.float32)
    attn_bias = np.asarray(attn_bias, dtype=np.float32)
    wq = np.asarray(wq, dtype=np.float32)
    wk = np.asarray(wk, dtype=np.float32)
    wv = np.asarray(wv, dtype=np.float32)
    wo = np.asarray(wo, dtype=np.float32)

    bias = attn_bias
    if mask.any():
        bias = bias + mask  # broadcast [1,1,S,S] over [B,H,S,S]

    nc = _get_nc()

    in_maps = []
    for core in range(NCORES):
        b, hg = divmod(core, HG)
        c0, c1 = hg * DG, (hg + 1) * DG
        m = {
            "xT": np.ascontiguousarray(x[b].T).astype(BF16),
            "wqT": np.ascontiguousarray(wq[c0:c1, :].T).astype(BF16),
            "wkT": np.ascontiguousarray(wk[c0:c1, :].T).astype(BF16),
            "wvT": np.ascontiguousarray(wv[c0:c1, :].T).astype(BF16),
            "woT": np.ascontiguousarray(wo[:, c0:c1].T).astype(BF16),
            # bias^T per local head: [h, k, q]
            "ebT": np.ascontiguousarray(
                bias[b, hg * HPG:(hg + 1) * HPG].transpose(0, 2, 1)
            ).astype(BF16),
        }
        in_maps.append(m)

    res = run_bass_kernel_spmd(nc, in_maps, core_ids=list(range(NCORES)))

    full = np.zeros((B, S, D), dtype=np.float32)
    for core in range(NCORES):
        b = core // HG
        full[b] += np.asarray(res.results[core]["out"], dtype=np.float32)
    return full


# revision 12
# speedup vs baseline: 1.0920x; 1.0166x over previous
"""Distributed Trainium2 (Bass/Tile) kernel for a 16-head attention block.

Problem: x:[2,2048,1024], 16 heads of dim 64, full [B,H,S,S] additive bias,
softmax, out-projection.  Runs SPMD on 8 NeuronCores: mesh = batch(2) x
head-group(4), i.e. each core handles one batch element and 4 heads
(tensor-parallel split of wq/wk/wv columns and wo rows).  Each core emits a
partial [S, D] output; the host sums the 4 head-group partials per batch.

Device-side formulation (per core, heads h=0..3 local):
  QT/KT = (w @ x^T) in [dq, s] layout, duplicated into both partition
          halves so K=64 score matmuls for adjacent k-tiles can run
          CONCURRENTLY on disjoint PE row-groups (tile_position packing).
  PSUM  = bias^T + K Q^T   (bias lands via TensorE identity-matmul for
          even k-tiles and via a VectorE add for odd k-tiles -- static
          load balance between the two engines)
  P^T   = exp(PSUM)                (one wide ScalarE op per [128,1024] tile)
  attnoutT/denoms via one matmul against V augmented with a ones column
  attnT = attnoutT * (1/denom) broadcast    (softmax normalization)
  out_partial = attnT^T @ woT     (TensorE, overlapped with the next
          query-half's attention)
All matmuls in bf16 with f32 PSUM accumulation; softmax math in f32.
"""

import os
import sys

try:
    import concourse  # noqa: F401
except ImportError:  # pragma: no cover - fallback for bare containers
    for _p in ("/opt/trn_rl_repo", os.path.expanduser("~/.axon_site/_ro/trn_rl_repo")):
        if os.path.isdir(_p) and _p not in sys.path:
            sys.path.insert(0, _p)

from contextlib import ExitStack

import ml_dtypes
import numpy as np

import concourse.mybir as mybir
import concourse.tile as tile
from concourse import bacc
from concourse.bass_utils import run_bass_kernel_spmd
from concourse.masks import make_identity

BF16 = ml_dtypes.bfloat16

B, S, D = 2, 2048, 1024
H, HD = 16, 64
NCORES = 8
HG = 4                 # head groups (tensor-parallel factor)
HPG = H // HG          # heads per group = 4
DG = HPG * HD          # feature cols per group = 256
P = 128
KD = D // P            # contraction chunks for projections = 8
ST = S // P            # 128-row tiles along sequence = 16
NQB = S // 512         # 512-wide query blocks = 4

_CACHE = {}


def _build_nc():
    bf = mybir.dt.bfloat16
    f32 = mybir.dt.float32
    Exp = mybir.ActivationFunctionType.Exp
    mult = mybir.AluOpType.mult
    add = mybir.AluOpType.add

    nc = bacc.Bacc("TRN2", target_bir_lowering=False, debug=False,
                   num_devices=NCORES)

    xT = nc.dram_tensor("xT", [D, S], bf, kind="ExternalInput")
    wqT = nc.dram_tensor("wqT", [D, DG], bf, kind="ExternalInput")
    wkT = nc.dram_tensor("wkT", [D, DG], bf, kind="ExternalInput")
    wvT = nc.dram_tensor("wvT", [D, DG], bf, kind="ExternalInput")
    woT = nc.dram_tensor("woT", [DG, D], bf, kind="ExternalInput")
    ebT = nc.dram_tensor("ebT", [HPG, S, S], bf, kind="ExternalInput")
    out = nc.dram_tensor("out", [S, D], f32, kind="ExternalOutput")

    with tile.TileContext(nc) as tc, ExitStack() as ctx:
        const = ctx.enter_context(tc.tile_pool(name="const", bufs=1))
        eb_pool = ctx.enter_context(tc.tile_pool(name="eb", bufs=8))
        pt_pool = ctx.enter_context(tc.tile_pool(name="pt", bufs=8))
        ev_pool = ctx.enter_context(tc.tile_pool(name="ev", bufs=3))
        outsb = ctx.enter_context(tc.tile_pool(name="outsb", bufs=3))

        # ---- persistent SBUF tensors ----
        xT_sb = const.tile([P, KD, S], bf, tag="xT_sb")
        wqT_sb = const.tile([P, KD, DG], bf, tag="wqT_sb")
        wkT_sb = const.tile([P, KD, DG], bf, tag="wkT_sb")
        wvT_sb = const.tile([P, KD, DG], bf, tag="wvT_sb")
        woT_sb = const.tile([P, DG // P, D], bf, tag="woT_sb")
        # per-head Q^T/K^T duplicated into both partition halves
        QT_sb = const.tile([P, HPG, S], bf, tag="QT_sb")
        KT_sb = const.tile([P, HPG, S], bf, tag="KT_sb")
        # V with a ones column appended per head: [s, h*(HD+1)]
        Vg_sb = const.tile([P, ST, HPG * (HD + 1)], bf, tag="Vg_sb")
        attnT_sb = const.tile([P, DG // P, S], bf, tag="attnT_sb")
        ident_sb = const.tile([P, P], bf, tag="ident_sb")

        make_identity(nc, ident_sb[:])

        # split the big input DMAs across engines/queues for a faster ramp
        xTr = xT[:].rearrange("(o p) s -> p o s", p=P)
        for kc in range(KD):
            eng = (nc.sync, nc.gpsimd, nc.scalar)[kc % 3]
            eng.dma_start(out=xT_sb[:, kc, :], in_=xTr[:, kc, :])
        nc.sync.dma_start(out=wqT_sb[:], in_=wqT[:].rearrange("(o p) c -> p o c", p=P))
        nc.gpsimd.dma_start(out=wkT_sb[:], in_=wkT[:].rearrange("(o p) c -> p o c", p=P))
        nc.sync.dma_start(out=wvT_sb[:], in_=wvT[:].rearrange("(o p) c -> p o c", p=P))
        nc.scalar.dma_start(out=woT_sb[:], in_=woT[:].rearrange("(o p) d -> p o d", p=P))

        # ones columns for V augmentation (V copies below overwrite the rest)
        nc.vector.memset(Vg_sb[:], 1.0)

        # ---- projections (own PSUM pool, closed before attention) ----
        with ExitStack() as ph1:
            psum_q = ph1.enter_context(
                tc.tile_pool(name="psum_q", bufs=4, space="PSUM"))
            for w_sb, dst, scale in ((wqT_sb, QT_sb, 0.125), (wkT_sb, KT_sb, None)):
                for m in range(DG // P):
                    for n in range(NQB):
                        ps = psum_q.tile([P, 512], f32, tag="q")
                        for kc in range(KD):
                            nc.tensor.matmul(
                                ps[:],
                                lhsT=w_sb[:, kc, m * P:(m + 1) * P],
                                rhs=xT_sb[:, kc, n * 512:(n + 1) * 512],
                                start=(kc == 0), stop=(kc == KD - 1),
                            )
                        # psum rows 0:64 = head 2m, rows 64:128 = head 2m+1;
                        # write each head's block into BOTH partition halves
                        nsl = slice(n * 512, (n + 1) * 512)
                        for h2 in range(2):
                            src = ps[h2 * HD:(h2 + 1) * HD, :]
                            for half in range(2):
                                dsl = dst[half * HD:(half + 1) * HD, 2 * m + h2, nsl]
                                if scale is None:
                                    if half == 0:
                                        nc.vector.tensor_copy(out=dsl, in_=src)
                                    else:
                                        nc.scalar.copy(out=dsl, in_=src)
                                else:
                                    if half == 0:
                                        nc.vector.tensor_scalar_mul(dsl, src, scale)
                                    else:
                                        nc.scalar.mul(out=dsl, in_=src, mul=scale)

            for t in range(ST):
                ps = psum_q.tile([P, DG], f32, tag="q")
                for kc in range(KD):
                    nc.tensor.matmul(
                        ps[:],
                        lhsT=xT_sb[:, kc, t * P:(t + 1) * P],
                        rhs=wvT_sb[:, kc, :],
                        start=(kc == 0), stop=(kc == KD - 1),
                    )
                for h in range(HPG):
                    vout = Vg_sb[:, t, h * (HD + 1):h * (HD + 1) + HD]
                    vin = ps[:, h * HD:(h + 1) * HD]
                    if h % 2 == 0:
                        nc.vector.tensor_copy(out=vout, in_=vin)
                    else:
                        nc.scalar.copy(out=vout, in_=vin)

        # ---- attention + overlapped output projection ----
        with ExitStack() as ph2:
            psum_s = ph2.enter_context(
                tc.tile_pool(name="psum_s", bufs=3, space="PSUM"))
            psum_o = ph2.enter_context(
                tc.tile_pool(name="psum_o", bufs=2, space="PSUM"))

            def attn_pass(h, qbp):
                hp, h2 = divmod(h, 2)
                q0 = qbp * 1024
                po = {}
                for qb2 in range(2):
                    po[qb2] = psum_o.tile(
                        [P, 512], f32, tag="o", name=f"po_{qb2}")
                for ktp in range(ST // 2):
                    ps = {}
                    ebt = {}
                    for u in range(2):          # unit = one k-tile of 128
                        kt = 2 * ktp + u
                        ebt[u] = eb_pool.tile([P, 1024], bf, tag="eb", name=f"ebt_{u}")
                        dma_eng = nc.sync if u == 0 else nc.gpsimd
                        dma_eng.dma_start(
                            out=ebt[u][:],
                            in_=ebT[h, kt * P:(kt + 1) * P, q0:q0 + 1024])
                        ps[u] = psum_s.tile([P, 1024], f32, tag="s", name=f"ps_{u}")
                    # unit 0: bias via TensorE identity-matmul (group
                    # start); unit 1: bias via a VectorE add after its scores
                    for qb2 in range(2):
                        nc.tensor.matmul(
                            ps[0][:, qb2 * 512:(qb2 + 1) * 512],
                            lhsT=ident_sb[:],
                            rhs=ebt[0][:, qb2 * 512:(qb2 + 1) * 512],
                            start=True, stop=False,
                        )
                    # scores: unit 0 on PE rows 0-63, unit 1 on rows 64-127,
                    # interleaved so the two K=64 matmuls can run concurrently
                    for qb2 in range(2):
                        for u in range(2):
                            kt = 2 * ktp + u
                            hh = slice(u * HD, (u + 1) * HD)
                            nc.tensor.matmul(
                                ps[u][:, qb2 * 512:(qb2 + 1) * 512],
                                lhsT=KT_sb[hh, h, kt * P:(kt + 1) * P],
                                rhs=QT_sb[hh, h,
                                          q0 + qb2 * 512:q0 + (qb2 + 1) * 512],
                                start=(u == 1), stop=True,
                                tile_position=(u * HD, 0),
                            )
                    nc.vector.tensor_tensor(ps[1][:], ps[1][:], ebt[1][:], add)
                    pt = {}
                    for u in range(2):
                        pt[u] = pt_pool.tile([P, 1024], bf, tag="pt",
                                             name=f"pt_{u}")
                        nc.scalar.activation(pt[u][:], ps[u][:], Exp)
                    for u in range(2):
                        kt = 2 * ktp + u
                        for qb2 in range(2):
                            nc.tensor.matmul(
                                po[qb2][:HD + 1, :],
                                lhsT=Vg_sb[:, kt,
                                           h * (HD + 1):(h + 1) * (HD + 1)],
                                rhs=pt[u][:, qb2 * 512:(qb2 + 1) * 512],
                                start=(kt == 0), stop=(kt == ST - 1),
                            )
                # evacuate the PV accumulators with single quick copies so
                # the PSUM banks free up for the next pass immediately; the
                # normalization chain then runs off-critical-path from SBUF
                poc = ev_pool.tile([HD, 1024], f32, tag="poc")
                den = ev_pool.tile([1, 1024], f32, tag="den")
                for qb2 in range(2):
                    qsl = slice(qb2 * 512, (qb2 + 1) * 512)
                    nc.vector.tensor_copy(out=poc[:, qsl], in_=po[qb2][:HD, :])
                    # single-channel reads must land on partition 0: pull the
                    # denominator row straight out of PSUM
                    nc.vector.tensor_copy(out=den[:, qsl], in_=po[qb2][HD:HD + 1, :])
                rc = ev_pool.tile([1, 1024], f32, tag="rc")
                nc.vector.reciprocal_approx_fast(out=rc[:], in_=den[:])
                bc = ev_pool.tile([HD, 1024], f32, tag="bc")
                nc.gpsimd.partition_broadcast(bc[:], rc[:])
                nc.vector.tensor_tensor(
                    attnT_sb[h2 * HD:(h2 + 1) * HD, hp, q0:q0 + 1024],
                    poc[:HD, :], bc[:], mult)

            def oproj_half(qbp):
                # out rows for this query half; fat psum tile = both nb halves
                for st in range(qbp * (ST // 2), (qbp + 1) * (ST // 2)):
                    ps = psum_s.tile([P, 1024], f32, tag="s", name="ps_w")
                    for nb in range(2):
                        for c in range(DG // P):
                            nc.tensor.matmul(
                                ps[:, nb * 512:(nb + 1) * 512],
                                lhsT=attnT_sb[:, c, st * P:(st + 1) * P],
                                rhs=woT_sb[:, c, nb * 512:(nb + 1) * 512],
                                start=(c == 0), stop=(c == DG // P - 1),
                            )
                    ob = outsb.tile([P, D], f32, tag="ob")
                    nc.vector.tensor_copy(out=ob[:], in_=ps[:])
                    nc.gpsimd.dma_start(out=out[st * P:(st + 1) * P, :], in_=ob[:])

            for qbp in range(2):
                for h in range(HPG):
                    attn_pass(h, qbp)
                oproj_half(qbp)

    nc.compile()
    return nc


def _get_nc():
    if "nc" not in _CACHE:
        _CACHE["nc"] = _build_nc()
    return _CACHE["nc"]


def kernel(x, mask, attn_bias, wq, wk, wv, wo):
    x = np.asarray(x, dtype=np.float32)
    mask = np.asarray(mask, dtype=np.float32)
    attn_bias = np.asarray(attn_bias, dtype=np.float32)
    wq = np.asarray(wq, dtype=np.float32)
    wk = np.asarray(wk, dtype=np.float32)
    wv = np.asarray(wv, dtype=np.float32)
    wo = np.asarray(wo, dtype=np.float32)

    bias = attn_bias
    if mask.any():
        bias = bias + mask  # broadcast [1,1,S,S] over [B,H,S,S]

    nc = _get_nc()

    in_maps = []
    for core in range(NCORES):
        b, hg = divmod(core, HG)
        c0, c1 = hg * DG, (hg + 1) * DG
        m = {
            "xT": np.ascontiguousarray(x[b].T).astype(BF16),
            "wqT": np.ascontiguousarray(wq[c0:c1, :].T).astype(BF16),
            "wkT": np.ascontiguousarray(wk[c0:c1, :].T).astype(BF16),
            "wvT": np.ascontiguousarray(wv[c0:c1, :].T).astype(BF16),
            "woT": np.ascontiguousarray(wo[:, c0:c1].T).astype(BF16),
            # bias^T per local head: [h, k, q]
            "ebT": np.ascontiguousarray(
                bias[b, hg * HPG:(hg + 1) * HPG].transpose(0, 2, 1)
            ).astype(BF16),
        }
        in_maps.append(m)

    res = run_bass_kernel_spmd(nc, in_maps, core_ids=list(range(NCORES)))

    full = np.zeros((B, S, D), dtype=np.float32)
    for core in range(NCORES):
        b = core // HG
        full[b] += np.asarray(res.results[core]["out"], dtype=np.float32)
    return full


# revision 13
# speedup vs baseline: 1.1061x; 1.0129x over previous
"""Distributed Trainium2 (Bass/Tile) kernel for a 16-head attention block.

Problem: x:[2,2048,1024], 16 heads of dim 64, full [B,H,S,S] additive bias,
softmax, out-projection.  Runs SPMD on 8 NeuronCores: mesh = batch(2) x
head-group(4), i.e. each core handles one batch element and 4 heads
(tensor-parallel split of wq/wk/wv columns and wo rows).  Each core emits a
partial [S, D] output; the host sums the 4 head-group partials per batch.

Device-side formulation (per core, heads h=0..3 local):
  QT/KT = (w @ x^T) in [dq, s] layout, duplicated into both partition
          halves so K=64 score matmuls for adjacent k-tiles can run
          CONCURRENTLY on disjoint PE row-groups (tile_position packing).
  PSUM  = bias^T + K Q^T   (bias lands via TensorE identity-matmul for
          even k-tiles and via a VectorE add for odd k-tiles -- static
          load balance between the two engines)
  P^T   = exp(PSUM)                (one wide ScalarE op per [128,1024] tile)
  attnoutT/denoms via one matmul against V augmented with a ones column
  attnT = attnoutT * (1/denom) broadcast    (softmax normalization)
  out_partial = attnT^T @ woT     (TensorE, overlapped with the next
          query-half's attention)
All matmuls in bf16 with f32 PSUM accumulation; softmax math in f32.
"""

import os
import sys

try:
    import concourse  # noqa: F401
except ImportError:  # pragma: no cover - fallback for bare containers
    for _p in ("/opt/trn_rl_repo", os.path.expanduser("~/.axon_site/_ro/trn_rl_repo")):
        if os.path.isdir(_p) and _p not in sys.path:
            sys.path.insert(0, _p)

from contextlib import ExitStack

import ml_dtypes
import numpy as np

import concourse.mybir as mybir
import concourse.tile as tile
from concourse import bacc
from concourse.bass_utils import run_bass_kernel_spmd
from concourse.masks import make_identity

BF16 = ml_dtypes.bfloat16

B, S, D = 2, 2048, 1024
H, HD = 16, 64
NCORES = 8
HG = 4                 # head groups (tensor-parallel factor)
HPG = H // HG          # heads per group = 4
DG = HPG * HD          # feature cols per group = 256
P = 128
KD = D // P            # contraction chunks for projections = 8
ST = S // P            # 128-row tiles along sequence = 16
NQB = S // 512         # 512-wide query blocks = 4

_CACHE = {}


def _build_nc():
    bf = mybir.dt.bfloat16
    f32 = mybir.dt.float32
    Exp = mybir.ActivationFunctionType.Exp
    mult = mybir.AluOpType.mult
    add = mybir.AluOpType.add

    nc = bacc.Bacc("TRN2", target_bir_lowering=False, debug=False,
                   num_devices=NCORES)

    xT = nc.dram_tensor("xT", [D, S], bf, kind="ExternalInput")
    wqT = nc.dram_tensor("wqT", [D, DG], bf, kind="ExternalInput")
    wkT = nc.dram_tensor("wkT", [D, DG], bf, kind="ExternalInput")
    wvT = nc.dram_tensor("wvT", [D, DG], bf, kind="ExternalInput")
    woT = nc.dram_tensor("woT", [DG, D], bf, kind="ExternalInput")
    ebT = nc.dram_tensor("ebT", [HPG, S, S], bf, kind="ExternalInput")
    out = nc.dram_tensor("out", [S, D], f32, kind="ExternalOutput")

    with tile.TileContext(nc) as tc, ExitStack() as ctx:
        const = ctx.enter_context(tc.tile_pool(name="const", bufs=1))
        eb_pool = ctx.enter_context(tc.tile_pool(name="eb", bufs=10))
        pt_pool = ctx.enter_context(tc.tile_pool(name="pt", bufs=8))
        ev_pool = ctx.enter_context(tc.tile_pool(name="ev", bufs=3))
        outsb = ctx.enter_context(tc.tile_pool(name="outsb", bufs=3))

        # ---- persistent SBUF tensors ----
        xT_sb = const.tile([P, KD, S], bf, tag="xT_sb")
        wqT_sb = const.tile([P, KD, DG], bf, tag="wqT_sb")
        wkT_sb = const.tile([P, KD, DG], bf, tag="wkT_sb")
        wvT_sb = const.tile([P, KD, DG], bf, tag="wvT_sb")
        woT_sb = const.tile([P, DG // P, D], bf, tag="woT_sb")
        # per-head Q^T/K^T duplicated into both partition halves
        QT_sb = const.tile([P, HPG, S], bf, tag="QT_sb")
        KT_sb = const.tile([P, HPG, S], bf, tag="KT_sb")
        # V with a ones column appended per head: [s, h*(HD+1)]
        Vg_sb = const.tile([P, ST, HPG * (HD + 1)], bf, tag="Vg_sb")
        attnT_sb = const.tile([P, DG // P, S], bf, tag="attnT_sb")
        ident_sb = const.tile([P, P], bf, tag="ident_sb")

        make_identity(nc, ident_sb[:])

        # split the big input DMAs across engines/queues for a faster ramp
        xTr = xT[:].rearrange("(o p) s -> p o s", p=P)
        for kc in range(KD):
            eng = (nc.sync, nc.gpsimd, nc.scalar)[kc % 3]
            eng.dma_start(out=xT_sb[:, kc, :], in_=xTr[:, kc, :])
        nc.sync.dma_start(out=wqT_sb[:], in_=wqT[:].rearrange("(o p) c -> p o c", p=P))
        nc.gpsimd.dma_start(out=wkT_sb[:], in_=wkT[:].rearrange("(o p) c -> p o c", p=P))
        nc.sync.dma_start(out=wvT_sb[:], in_=wvT[:].rearrange("(o p) c -> p o c", p=P))
        nc.scalar.dma_start(out=woT_sb[:], in_=woT[:].rearrange("(o p) d -> p o d", p=P))

        # ones columns for V augmentation (V copies below overwrite the rest)
        nc.vector.memset(Vg_sb[:], 1.0)

        # ---- projections (own PSUM pool, closed before attention) ----
        with ExitStack() as ph1:
            psum_q = ph1.enter_context(
                tc.tile_pool(name="psum_q", bufs=4, space="PSUM"))
            for t in range(ST):
                ps = psum_q.tile([P, DG], f32, tag="q")
                for kc in range(KD):
                    nc.tensor.matmul(
                        ps[:],
                        lhsT=xT_sb[:, kc, t * P:(t + 1) * P],
                        rhs=wvT_sb[:, kc, :],
                        start=(kc == 0), stop=(kc == KD - 1),
                    )
                for h in range(HPG):
                    vout = Vg_sb[:, t, h * (HD + 1):h * (HD + 1) + HD]
                    vin = ps[:, h * HD:(h + 1) * HD]
                    if h % 2 == 0:
                        nc.vector.tensor_copy(out=vout, in_=vin)
                    else:
                        nc.scalar.copy(out=vout, in_=vin)

            for w_sb, dst, scale in ((wqT_sb, QT_sb, 0.125), (wkT_sb, KT_sb, None)):
                for m in range(DG // P):
                    for n in range(NQB):
                        ps = psum_q.tile([P, 512], f32, tag="q")
                        for kc in range(KD):
                            nc.tensor.matmul(
                                ps[:],
                                lhsT=w_sb[:, kc, m * P:(m + 1) * P],
                                rhs=xT_sb[:, kc, n * 512:(n + 1) * 512],
                                start=(kc == 0), stop=(kc == KD - 1),
                            )
                        # psum rows 0:64 = head 2m, rows 64:128 = head 2m+1;
                        # write each head's block into BOTH partition halves
                        nsl = slice(n * 512, (n + 1) * 512)
                        for h2 in range(2):
                            src = ps[h2 * HD:(h2 + 1) * HD, :]
                            for half in range(2):
                                dsl = dst[half * HD:(half + 1) * HD, 2 * m + h2, nsl]
                                if scale is None:
                                    if half == 0:
                                        nc.vector.tensor_copy(out=dsl, in_=src)
                                    else:
                                        nc.scalar.copy(out=dsl, in_=src)
                                else:
                                    if half == 0:
                                        nc.vector.tensor_scalar_mul(dsl, src, scale)
                                    else:
                                        nc.scalar.mul(out=dsl, in_=src, mul=scale)


        # ---- attention + overlapped output projection ----
        with ExitStack() as ph2:
            psum_s = ph2.enter_context(
                tc.tile_pool(name="psum_s", bufs=3, space="PSUM"))
            psum_o = ph2.enter_context(
                tc.tile_pool(name="psum_o", bufs=2, space="PSUM"))

            def attn_pass(h, qbp):
                hp, h2 = divmod(h, 2)
                q0 = qbp * 1024
                po = {}
                for qb2 in range(2):
                    po[qb2] = psum_o.tile(
                        [P, 512], f32, tag="o", name=f"po_{qb2}")
                for ktp in range(ST // 2):
                    ps = {}
                    ebt = {}
                    for u in range(2):          # unit = one k-tile of 128
                        kt = 2 * ktp + u
                        ebt[u] = eb_pool.tile([P, 1024], bf, tag="eb", name=f"ebt_{u}")
                        dma_eng = nc.sync if u == 0 else nc.gpsimd
                        dma_eng.dma_start(
                            out=ebt[u][:],
                            in_=ebT[h, kt * P:(kt + 1) * P, q0:q0 + 1024])
                        ps[u] = psum_s.tile([P, 1024], f32, tag="s", name=f"ps_{u}")
                    # unit 0: bias via TensorE identity-matmul (group
                    # start); unit 1: bias via a VectorE add after its scores
                    for qb2 in range(2):
                        nc.tensor.matmul(
                            ps[0][:, qb2 * 512:(qb2 + 1) * 512],
                            lhsT=ident_sb[:],
                            rhs=ebt[0][:, qb2 * 512:(qb2 + 1) * 512],
                            start=True, stop=False,
                        )
                    # scores: unit 0 on PE rows 0-63, unit 1 on rows 64-127,
                    # interleaved so the two K=64 matmuls can run concurrently
                    for qb2 in range(2):
                        for u in range(2):
                            kt = 2 * ktp + u
                            hh = slice(u * HD, (u + 1) * HD)
                            nc.tensor.matmul(
                                ps[u][:, qb2 * 512:(qb2 + 1) * 512],
                                lhsT=KT_sb[hh, h, kt * P:(kt + 1) * P],
                                rhs=QT_sb[hh, h,
                                          q0 + qb2 * 512:q0 + (qb2 + 1) * 512],
                                start=(u == 1), stop=True,
                                tile_position=(u * HD, 0),
                            )
                    nc.vector.tensor_tensor(ps[1][:], ps[1][:], ebt[1][:], add)
                    pt = {}
                    for u in range(2):
                        pt[u] = pt_pool.tile([P, 1024], bf, tag="pt",
                                             name=f"pt_{u}")
                        nc.scalar.activation(pt[u][:], ps[u][:], Exp)
                    for u in range(2):
                        kt = 2 * ktp + u
                        for qb2 in range(2):
                            nc.tensor.matmul(
                                po[qb2][:HD + 1, :],
                                lhsT=Vg_sb[:, kt,
                                           h * (HD + 1):(h + 1) * (HD + 1)],
                                rhs=pt[u][:, qb2 * 512:(qb2 + 1) * 512],
                                start=(kt == 0), stop=(kt == ST - 1),
                            )
                # evacuate the PV accumulators with single quick copies so
                # the PSUM banks free up for the next pass immediately; the
                # normalization chain then runs off-critical-path from SBUF
                poc = ev_pool.tile([HD, 1024], f32, tag="poc")
                den = ev_pool.tile([1, 1024], f32, tag="den")
                for qb2 in range(2):
                    qsl = slice(qb2 * 512, (qb2 + 1) * 512)
                    nc.vector.tensor_copy(out=poc[:, qsl], in_=po[qb2][:HD, :])
                    # single-channel reads must land on partition 0: pull the
                    # denominator row straight out of PSUM
                    nc.vector.tensor_copy(out=den[:, qsl], in_=po[qb2][HD:HD + 1, :])
                rc = ev_pool.tile([1, 1024], f32, tag="rc")
                nc.vector.reciprocal_approx_fast(out=rc[:], in_=den[:])
                bc = ev_pool.tile([HD, 1024], f32, tag="bc")
                nc.gpsimd.partition_broadcast(bc[:], rc[:])
                nc.vector.tensor_tensor(
                    attnT_sb[h2 * HD:(h2 + 1) * HD, hp, q0:q0 + 1024],
                    poc[:HD, :], bc[:], mult)

            def oproj_half(qbp):
                # out rows for this query half; fat psum tile = both nb halves
                for st in range(qbp * (ST // 2), (qbp + 1) * (ST // 2)):
                    ps = psum_s.tile([P, 1024], f32, tag="s", name="ps_w")
                    for nb in range(2):
                        for c in range(DG // P):
                            nc.tensor.matmul(
                                ps[:, nb * 512:(nb + 1) * 512],
                                lhsT=attnT_sb[:, c, st * P:(st + 1) * P],
                                rhs=woT_sb[:, c, nb * 512:(nb + 1) * 512],
                                start=(c == 0), stop=(c == DG // P - 1),
                            )
                    ob = outsb.tile([P, D], f32, tag="ob")
                    nc.vector.tensor_copy(out=ob[:], in_=ps[:])
                    nc.gpsimd.dma_start(out=out[st * P:(st + 1) * P, :], in_=ob[:])

            for h in range(HPG):
                attn_pass(h, 0)
            attn_pass(0, 1)
            oproj_half(0)
            for h in range(1, HPG):
                attn_pass(h, 1)
            oproj_half(1)

    nc.compile()
    return nc


def _get_nc():
    if "nc" not in _CACHE:
        _CACHE["nc"] = _build_nc()
    return _CACHE["nc"]


def kernel(x, mask, attn_bias, wq, wk, wv, wo):
    x = np.asarray(x, dtype=np.float32)
    mask = np.asarray(mask, dtype=np.float32)
    attn_bias = np.asarray(attn_bias, dtype=np.float32)
    wq = np.asarray(wq, dtype=np.float32)
    wk = np.asarray(wk, dtype=np.float32)
    wv = np.asarray(wv, dtype=np.float32)
    wo = np.asarray(wo, dtype=np.float32)

    bias = attn_bias
    if mask.any():
        bias = bias + mask  # broadcast [1,1,S,S] over [B,H,S,S]

    nc = _get_nc()

    in_maps = []
    for core in range(NCORES):
        b, hg = divmod(core, HG)
        c0, c1 = hg * DG, (hg + 1) * DG
        m = {
            "xT": np.ascontiguousarray(x[b].T).astype(BF16),
            "wqT": np.ascontiguousarray(wq[c0:c1, :].T).astype(BF16),
            "wkT": np.ascontiguousarray(wk[c0:c1, :].T).astype(BF16),
            "wvT": np.ascontiguousarray(wv[c0:c1, :].T).astype(BF16),
            "woT": np.ascontiguousarray(wo[:, c0:c1].T).astype(BF16),
            # bias^T per local head: [h, k, q]
            "ebT": np.ascontiguousarray(
                bias[b, hg * HPG:(hg + 1) * HPG].transpose(0, 2, 1)
            ).astype(BF16),
        }
        in_maps.append(m)

    res = run_bass_kernel_spmd(nc, in_maps, core_ids=list(range(NCORES)))

    full = np.zeros((B, S, D), dtype=np.float32)
    for core in range(NCORES):
        b = core // HG
        full[b] += np.asarray(res.results[core]["out"], dtype=np.float32)
    return full


# revision 15
# speedup vs baseline: 1.1101x; 1.0036x over previous
"""Distributed Trainium2 (Bass/Tile) kernel for a 16-head attention block.

Problem: x:[2,2048,1024], 16 heads of dim 64, full [B,H,S,S] additive bias,
softmax, out-projection.  Runs SPMD on 8 NeuronCores: mesh = batch(2) x
head-group(4), i.e. each core handles one batch element and 4 heads
(tensor-parallel split of wq/wk/wv columns and wo rows).  Each core emits a
partial [S, D] output; the host sums the 4 head-group partials per batch.

Device-side formulation (per core, heads h=0..3 local):
  QT/KT = (w @ x^T) in [dq, s] layout, duplicated into both partition
          halves so K=64 score matmuls for adjacent k-tiles can run
          CONCURRENTLY on disjoint PE row-groups (tile_position packing).
  PSUM  = bias^T + K Q^T   (bias lands via TensorE identity-matmul for
          even k-tiles and via a VectorE add for odd k-tiles -- static
          load balance between the two engines)
  P^T   = exp(PSUM)                (one wide ScalarE op per [128,1024] tile)
  attnoutT/denoms via one matmul against V augmented with a ones column
  attnT = attnoutT * (1/denom) broadcast    (softmax normalization)
  out_partial = attnT^T @ woT     (TensorE, overlapped with the next
          query-half's attention)
All matmuls in bf16 with f32 PSUM accumulation; softmax math in f32.
"""

import os
import sys

try:
    import concourse  # noqa: F401
except ImportError:  # pragma: no cover - fallback for bare containers
    for _p in ("/opt/trn_rl_repo", os.path.expanduser("~/.axon_site/_ro/trn_rl_repo")):
        if os.path.isdir(_p) and _p not in sys.path:
            sys.path.insert(0, _p)

from contextlib import ExitStack

import ml_dtypes
import numpy as np

import concourse.mybir as mybir
import concourse.tile as tile
from concourse import bacc
from concourse.bass_utils import run_bass_kernel_spmd
from concourse.masks import make_identity

BF16 = ml_dtypes.bfloat16

B, S, D = 2, 2048, 1024
H, HD = 16, 64
NCORES = 8
HG = 4                 # head groups (tensor-parallel factor)
HPG = H // HG          # heads per group = 4
DG = HPG * HD          # feature cols per group = 256
P = 128
KD = D // P            # contraction chunks for projections = 8
ST = S // P            # 128-row tiles along sequence = 16
NQB = S // 512         # 512-wide query blocks = 4

_CACHE = {}


def _build_nc():
    bf = mybir.dt.bfloat16
    f32 = mybir.dt.float32
    Exp = mybir.ActivationFunctionType.Exp
    mult = mybir.AluOpType.mult
    add = mybir.AluOpType.add

    nc = bacc.Bacc("TRN2", target_bir_lowering=False, debug=False,
                   num_devices=NCORES)

    xT = nc.dram_tensor("xT", [D, S], bf, kind="ExternalInput")
    wqT = nc.dram_tensor("wqT", [D, DG], bf, kind="ExternalInput")
    wkT = nc.dram_tensor("wkT", [D, DG], bf, kind="ExternalInput")
    wvT = nc.dram_tensor("wvT", [D, DG], bf, kind="ExternalInput")
    woT = nc.dram_tensor("woT", [DG, D], bf, kind="ExternalInput")
    ebT = nc.dram_tensor("ebT", [HPG, S, S], bf, kind="ExternalInput")
    out = nc.dram_tensor("out", [S, D], f32, kind="ExternalOutput")

    with tile.TileContext(nc) as tc, ExitStack() as ctx:
        const = ctx.enter_context(tc.tile_pool(name="const", bufs=1))
        eb_pool = ctx.enter_context(tc.tile_pool(name="eb", bufs=10))
        pt_pool = ctx.enter_context(tc.tile_pool(name="pt", bufs=8))
        ev_pool = ctx.enter_context(tc.tile_pool(name="ev", bufs=3))
        outsb = ctx.enter_context(tc.tile_pool(name="outsb", bufs=3))

        # ---- persistent SBUF tensors ----
        xT_sb = const.tile([P, KD, S], bf, tag="xT_sb")
        wqT_sb = const.tile([P, KD, DG], bf, tag="wqT_sb")
        wkT_sb = const.tile([P, KD, DG], bf, tag="wkT_sb")
        wvT_sb = const.tile([P, KD, DG], bf, tag="wvT_sb")
        woT_sb = const.tile([P, DG // P, D], bf, tag="woT_sb")
        # per-head Q^T/K^T duplicated into both partition halves
        QT_sb = const.tile([P, HPG, S], bf, tag="QT_sb")
        KT_sb = const.tile([P, HPG, S], bf, tag="KT_sb")
        # V with a ones column appended per head: [s, h*(HD+1)]
        Vg_sb = const.tile([P, ST, HPG * (HD + 1)], bf, tag="Vg_sb")
        attnT_sb = const.tile([P, DG // P, S], bf, tag="attnT_sb")
        ident_sb = const.tile([P, P], bf, tag="ident_sb")

        make_identity(nc, ident_sb[:])

        # split the big input DMAs into many chunks across all three
        # DMA-capable engines so the transfers spread over many HW queues
        # (per-queue bandwidth is ~65 GB/s; the ramp needs ~4.5 MB)
        engs = (nc.sync, nc.gpsimd, nc.scalar)
        xTr = xT[:].rearrange("(o p) s -> p o s", p=P)
        ei = 0
        for kc in range(KD):
            for hlf in range(2):
                engs[ei % 3].dma_start(
                    out=xT_sb[:, kc, hlf * 1024:(hlf + 1) * 1024],
                    in_=xTr[:, kc, hlf * 1024:(hlf + 1) * 1024])
                ei += 1
        wvTr = wvT[:].rearrange("(o p) c -> p o c", p=P)
        for kc in range(KD):
            engs[ei % 3].dma_start(out=wvT_sb[:, kc, :], in_=wvTr[:, kc, :])
            ei += 1
        nc.sync.dma_start(out=wqT_sb[:], in_=wqT[:].rearrange("(o p) c -> p o c", p=P))
        nc.gpsimd.dma_start(out=wkT_sb[:], in_=wkT[:].rearrange("(o p) c -> p o c", p=P))
        nc.scalar.dma_start(out=woT_sb[:], in_=woT[:].rearrange("(o p) d -> p o d", p=P))

        # ones columns for V augmentation (V copies below overwrite the rest)
        nc.vector.memset(Vg_sb[:], 1.0)

        # ---- projections (own PSUM pool, closed before attention) ----
        with ExitStack() as ph1:
            psum_q = ph1.enter_context(
                tc.tile_pool(name="psum_q", bufs=4, space="PSUM"))
            for t in range(ST):
                ps = psum_q.tile([P, DG], f32, tag="q")
                for kc in range(KD):
                    nc.tensor.matmul(
                        ps[:],
                        lhsT=xT_sb[:, kc, t * P:(t + 1) * P],
                        rhs=wvT_sb[:, kc, :],
                        start=(kc == 0), stop=(kc == KD - 1),
                    )
                for h in range(HPG):
                    vout = Vg_sb[:, t, h * (HD + 1):h * (HD + 1) + HD]
                    vin = ps[:, h * HD:(h + 1) * HD]
                    if h % 2 == 0:
                        nc.vector.tensor_copy(out=vout, in_=vin)
                    else:
                        nc.scalar.copy(out=vout, in_=vin)

            for w_sb, dst, scale in ((wqT_sb, QT_sb, 0.125), (wkT_sb, KT_sb, None)):
                for m in range(DG // P):
                    for n in range(NQB):
                        ps = psum_q.tile([P, 512], f32, tag="q")
                        for kc in range(KD):
                            nc.tensor.matmul(
                                ps[:],
                                lhsT=w_sb[:, kc, m * P:(m + 1) * P],
                                rhs=xT_sb[:, kc, n * 512:(n + 1) * 512],
                                start=(kc == 0), stop=(kc == KD - 1),
                            )
                        # psum rows 0:64 = head 2m, rows 64:128 = head 2m+1;
                        # write each head's block into BOTH partition halves
                        nsl = slice(n * 512, (n + 1) * 512)
                        for h2 in range(2):
                            src = ps[h2 * HD:(h2 + 1) * HD, :]
                            for half in range(2):
                                dsl = dst[half * HD:(half + 1) * HD, 2 * m + h2, nsl]
                                if scale is None:
                                    if half == 0:
                                        nc.vector.tensor_copy(out=dsl, in_=src)
                                    else:
                                        nc.scalar.copy(out=dsl, in_=src)
                                else:
                                    if half == 0:
                                        nc.vector.tensor_scalar_mul(dsl, src, scale)
                                    else:
                                        nc.scalar.mul(out=dsl, in_=src, mul=scale)


        # ---- attention + overlapped output projection ----
        with ExitStack() as ph2:
            psum_s = ph2.enter_context(
                tc.tile_pool(name="psum_s", bufs=3, space="PSUM"))
            psum_o = ph2.enter_context(
                tc.tile_pool(name="psum_o", bufs=2, space="PSUM"))

            def attn_pass(h, qbp):
                hp, h2 = divmod(h, 2)
                q0 = qbp * 1024
                po = {}
                for qb2 in range(2):
                    po[qb2] = psum_o.tile(
                        [P, 512], f32, tag="o", name=f"po_{qb2}")
                for ktp in range(ST // 2):
                    ps = {}
                    ebt = {}
                    for u in range(2):          # unit = one k-tile of 128
                        kt = 2 * ktp + u
                        ebt[u] = eb_pool.tile([P, 1024], bf, tag="eb", name=f"ebt_{u}")
                        dma_eng = nc.sync if u == 0 else nc.gpsimd
                        dma_eng.dma_start(
                            out=ebt[u][:],
                            in_=ebT[h, kt * P:(kt + 1) * P, q0:q0 + 1024])
                        ps[u] = psum_s.tile([P, 1024], f32, tag="s", name=f"ps_{u}")
                    # unit 0: bias via TensorE identity-matmul (group
                    # start); unit 1: bias via a VectorE add after its scores
                    for qb2 in range(2):
                        nc.tensor.matmul(
                            ps[0][:, qb2 * 512:(qb2 + 1) * 512],
                            lhsT=ident_sb[:],
                            rhs=ebt[0][:, qb2 * 512:(qb2 + 1) * 512],
                            start=True, stop=False,
                        )
                    # scores: unit 0 on PE rows 0-63, unit 1 on rows 64-127,
                    # interleaved so the two K=64 matmuls can run concurrently
                    for qb2 in range(2):
                        for u in range(2):
                            kt = 2 * ktp + u
                            hh = slice(u * HD, (u + 1) * HD)
                            nc.tensor.matmul(
                                ps[u][:, qb2 * 512:(qb2 + 1) * 512],
                                lhsT=KT_sb[hh, h, kt * P:(kt + 1) * P],
                                rhs=QT_sb[hh, h,
                                          q0 + qb2 * 512:q0 + (qb2 + 1) * 512],
                                start=(u == 1), stop=True,
                                tile_position=(u * HD, 0),
                            )
                    nc.vector.tensor_tensor(ps[1][:], ps[1][:], ebt[1][:], add)
                    pt = {}
                    for u in range(2):
                        pt[u] = pt_pool.tile([P, 1024], bf, tag="pt",
                                             name=f"pt_{u}")
                        nc.scalar.activation(pt[u][:], ps[u][:], Exp)
                    for u in range(2):
                        kt = 2 * ktp + u
                        for qb2 in range(2):
                            nc.tensor.matmul(
                                po[qb2][:HD + 1, :],
                                lhsT=Vg_sb[:, kt,
                                           h * (HD + 1):(h + 1) * (HD + 1)],
                                rhs=pt[u][:, qb2 * 512:(qb2 + 1) * 512],
                                start=(kt == 0), stop=(kt == ST - 1),
                            )
                # evacuate the PV accumulators with single quick copies so
                # the PSUM banks free up for the next pass immediately; the
                # normalization chain then runs off-critical-path from SBUF
                poc = ev_pool.tile([HD, 1024], f32, tag="poc")
                den = ev_pool.tile([1, 1024], f32, tag="den")
                for qb2 in range(2):
                    qsl = slice(qb2 * 512, (qb2 + 1) * 512)
                    nc.vector.tensor_copy(out=poc[:, qsl], in_=po[qb2][:HD, :])
                    # single-channel reads must land on partition 0: pull the
                    # denominator row straight out of PSUM
                    nc.vector.tensor_copy(out=den[:, qsl], in_=po[qb2][HD:HD + 1, :])
                rc = ev_pool.tile([1, 1024], f32, tag="rc")
                nc.vector.reciprocal_approx_fast(out=rc[:], in_=den[:])
                bc = ev_pool.tile([HD, 1024], f32, tag="bc")
                nc.gpsimd.partition_broadcast(bc[:], rc[:])
                nc.vector.tensor_tensor(
                    attnT_sb[h2 * HD:(h2 + 1) * HD, hp, q0:q0 + 1024],
                    poc[:HD, :], bc[:], mult)

            def oproj_half(qbp):
                # out rows for this query half; fat psum tile = both nb halves
                for st in range(qbp * (ST // 2), (qbp + 1) * (ST // 2)):
                    ps = psum_s.tile([P, 1024], f32, tag="s", name="ps_w")
                    for nb in range(2):
                        for c in range(DG // P):
                            nc.tensor.matmul(
                                ps[:, nb * 512:(nb + 1) * 512],
                                lhsT=attnT_sb[:, c, st * P:(st + 1) * P],
                                rhs=woT_sb[:, c, nb * 512:(nb + 1) * 512],
                                start=(c == 0), stop=(c == DG // P - 1),
                            )
                    ob = outsb.tile([P, D], f32, tag="ob")
                    nc.vector.tensor_copy(out=ob[:], in_=ps[:])
                    nc.gpsimd.dma_start(out=out[st * P:(st + 1) * P, :], in_=ob[:])

            for h in range(HPG):
                attn_pass(h, 0)
            attn_pass(0, 1)
            oproj_half(0)
            for h in range(1, HPG):
                attn_pass(h, 1)
            oproj_half(1)

    nc.compile()
    return nc


def _get_nc():
    if "nc" not in _CACHE:
        _CACHE["nc"] = _build_nc()
    return _CACHE["nc"]


def kernel(x, mask, attn_bias, wq, wk, wv, wo):
    x = np.asarray(x, dtype=np.float32)
    mask = np.asarray(mask, dtype=np.float32)
    attn_bias = np.asarray(attn_bias, dtype=np.float32)
    wq = np.asarray(wq, dtype=np.float32)
    wk = np.asarray(wk, dtype=np.float32)
    wv = np.asarray(wv, dtype=np.float32)
    wo = np.asarray(wo, dtype=np.float32)

    bias = attn_bias
    if mask.any():
        bias = bias + mask  # broadcast [1,1,S,S] over [B,H,S,S]

    nc = _get_nc()

    in_maps = []
    for core in range(NCORES):
        b, hg = divmod(core, HG)
        c0, c1 = hg * DG, (hg + 1) * DG
        m = {
            "xT": np.ascontiguousarray(x[b].T).astype(BF16),
            "wqT": np.ascontiguousarray(wq[c0:c1, :].T).astype(BF16),
            "wkT": np.ascontiguousarray(wk[c0:c1, :].T).astype(BF16),
            "wvT": np.ascontiguousarray(wv[c0:c1, :].T).astype(BF16),
            "woT": np.ascontiguousarray(wo[:, c0:c1].T).astype(BF16),
            # bias^T per local head: [h, k, q]
            "ebT": np.ascontiguousarray(
                bias[b, hg * HPG:(hg + 1) * HPG].transpose(0, 2, 1)
            ).astype(BF16),
        }
        in_maps.append(m)

    res = run_bass_kernel_spmd(nc, in_maps, core_ids=list(range(NCORES)))

    full = np.zeros((B, S, D), dtype=np.float32)
    for core in range(NCORES):
        b = core // HG
        full[b] += np.asarray(res.results[core]["out"], dtype=np.float32)
    return full
